# revision 31
# baseline (speedup 1.0000x reference)
"""Trainium2 Bass kernel for the FEM dual-attention module.

Full (unsharded) inputs in, full outputs (E_q, E_s) out. Internally:
data-parallel over batch B=16 across 8 NeuronCores (2 samples each); the
BatchNorm batch statistics are combined with a tiny in-kernel AllReduce.

Self-contained: hardcodes all shapes; imports only concourse + numpy.
"""

import numpy as np

import concourse.bass as bass
import concourse.mybir as mybir
import concourse.tile as tile
from concourse import bacc
from concourse.bass_utils import run_bass_kernel_spmd
from concourse.masks import make_identity

# Problem shapes (hardcoded per spec)
B, C, N, IC, R = 16, 320, 4096, 128, 4
EPS = 1e-5
NCORES = 8
BPC = B // NCORES            # samples per core = 2
P = 128                      # SBUF partitions
NT = N // 512                # 8 n-tiles of 512 tokens
CCH = [(0, 128), (128, 128), (256, 64)]  # channel chunks of C=320
F32 = mybir.dt.float32
F32R = mybir.dt.float32r
BF16 = mybir.dt.bfloat16
ROWS_TOTAL = float(B * N)    # BN row count (global)
AX = mybir.AxisListType.X
AF = mybir.ActivationFunctionType

# P24 column map: per-channel vectors packed [128, 3] each (chunk-major)
COL_SCALE_P, COL_SHIFT_P, COL_SCALE_Q, COL_SHIFT_Q = 0, 3, 6, 9
COL_GATE_S = [12, 18]   # per sample b: gate from s
COL_GATE_Q = [15, 21]   # per sample b: gate from q

_CACHE = {}


def build_program(reps=1):
    """Build the SPMD program. reps>1 repeats the whole body (for timing
    calibration: per-rep time = (t[K] - t[1]) / (K - 1))."""
    nc = bacc.Bacc("TRN2", target_bir_lowering=False, debug=False,
                   num_devices=NCORES)

    # ---- DRAM I/O ----
    q_loc = nc.dram_tensor("q_loc", [BPC, C, N], F32, kind="ExternalInput").ap()
    s_loc = nc.dram_tensor("s_loc", [BPC, C, N], F32, kind="ExternalInput").ap()
    Wv = nc.dram_tensor("Wv", [C, IC], F32, kind="ExternalInput").ap()
    bv = nc.dram_tensor("bv", [IC], F32, kind="ExternalInput").ap()
    Wk = nc.dram_tensor("Wk", [C, IC], F32, kind="ExternalInput").ap()
    bk = nc.dram_tensor("bk", [IC], F32, kind="ExternalInput").ap()
    Wqp = nc.dram_tensor("Wqp", [C, IC], F32, kind="ExternalInput").ap()
    bqp = nc.dram_tensor("bqp", [IC], F32, kind="ExternalInput").ap()
    Wts = nc.dram_tensor("Wts", [IC, C], F32, kind="ExternalInput").ap()
    Wtq = nc.dram_tensor("Wtq", [IC, C], F32, kind="ExternalInput").ap()
    gts = nc.dram_tensor("gts", [C], F32, kind="ExternalInput").ap()
    bets = nc.dram_tensor("bets", [C], F32, kind="ExternalInput").ap()
    gtq = nc.dram_tensor("gtq", [C], F32, kind="ExternalInput").ap()
    betq = nc.dram_tensor("betq", [C], F32, kind="ExternalInput").ap()
    Wg1 = nc.dram_tensor("Wg1", [C, C // R], F32, kind="ExternalInput").ap()
    bg1 = nc.dram_tensor("bg1", [C // R], F32, kind="ExternalInput").ap()
    Wg2 = nc.dram_tensor("Wg2", [C // R, C], F32, kind="ExternalInput").ap()
    bg2 = nc.dram_tensor("bg2", [C], F32, kind="ExternalInput").ap()
    eq_loc = nc.dram_tensor("eq_loc", [BPC, C, N], F32, kind="ExternalOutput").ap()
    es_loc = nc.dram_tensor("es_loc", [BPC, C, N], F32, kind="ExternalOutput").ap()

    G = C // R  # 80

    with tile.TileContext(nc) as tc:
        nc._lp_ctx = nc.allow_low_precision(
            reason="float32r matmul operands (same 4-byte width as float32)")
        nc._lp_ctx.__enter__()
        with (
            tc.tile_pool(name="singles", bufs=1) as singles,
            tc.tile_pool(name="resident", bufs=2) as resident,   # per-sample, both live
            tc.tile_pool(name="work", bufs=2) as work,
            tc.tile_pool(name="rext", bufs=1) as rext,           # big transient panels
            tc.tile_pool(name="stream_in", bufs=2) as stream_in, # input n-tiles
            tc.tile_pool(name="stream", bufs=2) as stream,       # small per-ntile tiles
            tc.tile_pool(name="atts", bufs=2) as atts,           # attention 128x128s
            tc.tile_pool(name="smalls", bufs=4) as smalls,
            tc.tile_pool(name="mid", bufs=2) as mid,
            tc.tile_pool(name="ps_big", bufs=4, space="PSUM") as ps_big,
            tc.tile_pool(name="ps_small", bufs=4, space="PSUM") as ps_small,
            tc.tile_pool(name="dram", bufs=1, space="DRAM") as dram,
        ):
            # ---------- load weights / constants ----------
            def load_kxm(w_ap, m, dt=F32R):
                t = singles.tile([P, 3, m], dt, tag=f"w_{w_ap.name}")
                nc.sync.dma_start(
                    t[:, 0:2, :],
                    w_ap[0:256, :].rearrange("(o p) i -> p o i", p=P).bitcast(dt))
                nc.sync.dma_start(t[:64, 2, :], w_ap[256:C, :].bitcast(dt))
                return t

            def load_cvec(v_ap):
                t = singles.tile([P, 3], F32, tag=f"v_{v_ap.name}")
                nc.vector.memset(t[:], 0.0)
                nc.sync.dma_start(
                    t[:, 0:2], v_ap[0:256].rearrange("(o p) -> p o", p=P))
                nc.sync.dma_start(t[:64, 2:3], v_ap[256:C].unsqueeze(1))
                return t

            Wv_t = load_kxm(Wv, IC)
            Wk_t = load_kxm(Wk, IC)
            Wq_t = load_kxm(Wqp, IC)
            Wg1_t = load_kxm(Wg1, G, dt=F32)
            Wts_t = singles.tile([P, C], F32R, tag="wts")
            nc.sync.dma_start(Wts_t[:], Wts[:, :].bitcast(F32R))
            Wtq_t = singles.tile([P, C], F32R, tag="wtq")
            nc.sync.dma_start(Wtq_t[:], Wtq[:, :].bitcast(F32R))
            Wg2_t = singles.tile([G, C], F32, tag="wg2")
            nc.sync.dma_start(Wg2_t[:], Wg2[:, :])

            bv_t = singles.tile([P, 1], F32, tag="bv")
            nc.sync.dma_start(bv_t[:], bv.unsqueeze(1))
            bk_t = singles.tile([P, 1], F32, tag="bk")
            nc.sync.dma_start(bk_t[:], bk.unsqueeze(1))
            bq_t = singles.tile([P, 1], F32, tag="bq")
            nc.sync.dma_start(bq_t[:], bqp.unsqueeze(1))
            bg1_t = singles.tile([G, 1], F32, tag="bg1")
            nc.sync.dma_start(bg1_t[:], bg1.unsqueeze(1))

            gts_t = load_cvec(gts)
            bets_t = load_cvec(bets)
            gtq_t = load_cvec(gtq)
            betq_t = load_cvec(betq)
            bg2_t = load_cvec(bg2)

            ident = singles.tile([P, P], F32, tag="ident")
            make_identity(nc, ident[:])
            eps_t = singles.tile([P, 1], F32, tag="eps")
            nc.vector.memset(eps_t[:], EPS)
            ones_bf = singles.tile([1, 512], BF16, tag="ones_bf")
            nc.vector.memset(ones_bf[:], 1.0)

            WORK_TAGS = ["wk_a", "wk_b", "wk_c", "wk_d"]

            def emit_body():
                # BN sums accumulator: cols [sumP(3) ssqP(3) sumQ(3) ssqQ(3)]
                acc = smalls.tile([P, 12], F32, tag="acc")
                nc.vector.memset(acc[:], 0.0)
                # packed per-channel vectors (see COL_* map)
                P24 = smalls.tile([P, 24], F32, tag="p24")
                nc.vector.memset(P24[:], 0.0)

                p_sbs, q_sbs = [], []

                # ================= PHASE 1 =================
                for b in range(BPC):
                    v_s = work.tile([P, NT, 512], F32R, tag="wv_s")
                    v_q = work.tile([P, NT, 512], F32R, tag="wv_q")
                    # A_s / A_s^T accumulate across the whole token stream
                    psA = ps_small.tile([P, 512], F32, tag="pss", name="psA")[:, :P]
                    psAT = ps_small.tile([P, 512], F32, tag="pss", name="psAT")[:, :P]

                    for nt in range(NT):
                        ns = slice(nt * 512, (nt + 1) * 512)
                        in_q = stream_in.tile([P, 3, 512], F32R, tag="in_q")
                        in_s = stream_in.tile([P, 3, 512], F32R, tag="in_s")
                        for srcd, dst in ((q_loc, in_q), (s_loc, in_s)):
                            nc.sync.dma_start(
                                dst[:, 0:2, :],
                                srcd[b, 0:256, ns]
                                .rearrange("(o p) n -> p o n", p=P).bitcast(F32R))
                            nc.sync.dma_start(
                                dst[:64, 2, :], srcd[b, 256:C, ns].bitcast(F32R))

                        def proj(w_t, in_t):
                            ps = ps_big.tile([P, 512], F32, tag="ps")
                            for o, (c0, pc) in enumerate(CCH):
                                nc.tensor.matmul(ps[:], w_t[:pc, o, :],
                                                 in_t[:pc, o, :],
                                                 start=(o == 0), stop=(o == 2))
                            return ps

                        # v projections: PSUM->SBUF + bias on DVE
                        ps = proj(Wv_t, in_s)
                        nc.vector.tensor_scalar_add(v_s[:, nt, :], ps[:], bv_t[:])
                        ps = proj(Wv_t, in_q)
                        nc.vector.tensor_scalar_add(v_q[:, nt, :], ps[:], bv_t[:])
                        # k/q projections: PSUM->SBUF + bias on ACT
                        kx = stream.tile([P, 512], F32, tag="kx")
                        qx = stream.tile([P, 512], F32, tag="qx")
                        ps = proj(Wk_t, in_s)
                        nc.scalar.activation(kx[:], ps[:], AF.Identity,
                                             bias=bk_t[:], scale=1.0)
                        ps = proj(Wq_t, in_q)
                        nc.scalar.activation(qx[:], ps[:], AF.Identity,
                                             bias=bq_t[:], scale=1.0)

                        # transpose the four 128-token chunks of k/q, then
                        # immediately fold them into the A / A^T accumulators
                        ptk = ps_small.tile([P, 512], F32, tag="pss")
                        ptq = ps_small.tile([P, 512], F32, tag="pss")
                        for u in range(4):
                            us = slice(u * P, (u + 1) * P)
                            nc.tensor.transpose(ptk[:, us], kx[:, us], ident[:])
                            nc.tensor.transpose(ptq[:, us], qx[:, us], ident[:])
                        kTc = stream.tile([P, 4, P], F32R, tag="kTc")
                        qTc = stream.tile([P, 4, P], F32R, tag="qTc")
                        nc.scalar.copy(kTc[:].rearrange("p a b -> p (a b)"), ptk[:])
                        nc.scalar.copy(qTc[:].rearrange("p a b -> p (a b)"), ptq[:])
                        for u in range(4):
                            nc.tensor.matmul(psA[:], kTc[:, u, :], qTc[:, u, :],
                                             start=(nt == 0 and u == 0),
                                             stop=(nt == NT - 1 and u == 3))
                            nc.tensor.matmul(psAT[:], qTc[:, u, :], kTc[:, u, :],
                                             start=(nt == 0 and u == 0),
                                             stop=(nt == NT - 1 and u == 3))

                    def softmax_exp(psX, tag):
                        negm = smalls.tile([P, 1], F32, tag=f"negm_{tag}")
                        nc.vector.reduce_max(negm[:], psX[:], axis=AX, negate=True)
                        e = atts.tile([P, P], F32, tag=f"e_{tag}")
                        nc.scalar.activation(e[:], psX[:], AF.Exp,
                                             bias=negm[:], scale=1.0)
                        ssum = smalls.tile([P, 1], F32, tag=f"ssum_{tag}")
                        nc.vector.reduce_sum(ssum[:], e[:], axis=AX)
                        rinv = smalls.tile([P, 1], F32, tag=f"rinv_{tag}")
                        nc.vector.reciprocal(rinv[:], ssum[:])
                        return e, rinv

                    e_s, rinv_s = softmax_exp(psA, "s")
                    e_q, rinv_q = softmax_exp(psAT, "q")

                    def transpose_sb(src, tag):
                        pt = ps_small.tile([P, P], F32, tag="pss")
                        nc.tensor.transpose(pt[:], src[:], ident[:])
                        dst = atts.tile([P, P], F32R, tag=tag)
                        nc.scalar.copy(dst[:], pt[:])
                        return dst

                    eT_s = transpose_sb(e_s, "eT_s")
                    eT_q = transpose_sb(e_q, "eT_q")

                    # p_s = att_s @ v_s ; q_s = att_q @ v_q (row-scale rinv on ACT)
                    p_sb = resident.tile([P, NT, 512], F32R, tag="p_sb")
                    q_sb = resident.tile([P, NT, 512], F32R, tag="q_sb")
                    for nt in range(NT):
                        pp = ps_big.tile([P, 512], F32, tag="ps")
                        nc.tensor.matmul(pp[:], eT_s[:], v_s[:, nt, :])
                        nc.vector.tensor_scalar_mul(p_sb[:, nt, :], pp[:], rinv_s[:])
                        pq = ps_big.tile([P, 512], F32, tag="ps")
                        nc.tensor.matmul(pq[:], eT_q[:], v_q[:, nt, :])
                        nc.vector.tensor_scalar_mul(q_sb[:, nt, :], pq[:], rinv_q[:])
                    p_sbs.append(p_sb)
                    q_sbs.append(q_sb)

                    # ---- BN statistics of t^T = W^T @ p (linear bias cancels):
                    # per chunk: sum via DVE reduce, sum-of-squares via ACT Square
                    for srcT, w_t, col in ((p_sb, Wts_t, 0), (q_sb, Wtq_t, 6)):
                        for o, (c0, pc) in enumerate(CCH):
                            ssum = smalls.tile([P, NT], F32, tag="st_sum")
                            sssq = smalls.tile([P, NT], F32, tag="st_ssq")
                            for nt in range(NT):
                                pt = ps_big.tile([P, 512], F32, tag="ps")
                                nc.tensor.matmul(pt[:pc, :], w_t[:, c0:c0 + pc],
                                                 srcT[:, nt, :])
                                nc.vector.reduce_sum(ssum[:pc, nt:nt + 1],
                                                     pt[:pc, :], axis=AX)
                                junk3 = mid.tile([P, 512], F32, tag="junk")
                                nc.scalar.activation(
                                    junk3[:pc, :], pt[:pc, :], AF.Square,
                                    accum_out=sssq[:pc, nt:nt + 1])
                            tmp = smalls.tile([P, 2], F32, tag="bn_tmp")
                            nc.vector.reduce_sum(tmp[:pc, 0:1], ssum[:pc, :],
                                                 axis=AX)
                            nc.vector.reduce_sum(tmp[:pc, 1:2], sssq[:pc, :],
                                                 axis=AX)
                            nc.vector.tensor_add(acc[:pc, col + o:col + 1 + o],
                                                 acc[:pc, col + o:col + 1 + o],
                                                 tmp[:pc, 0:1])
                            nc.vector.tensor_add(acc[:pc, col + 3 + o:col + 4 + o],
                                                 acc[:pc, col + 3 + o:col + 4 + o],
                                                 tmp[:pc, 1:2])

                # ================= PHASE 2 + 3 =================
                # Residual panels double as the channel-gate input: pooled
                # means and the gate MLPs run on the re-loaded panels, which
                # overlap the AllReduce.
                pans = [
                    (b, path, *rest)
                    for b in range(BPC)
                    for path, rest in enumerate((
                        (p_sbs[b], Wts_t, COL_SCALE_P, COL_SHIFT_P,
                         COL_GATE_S[b], s_loc, es_loc),
                        (q_sbs[b], Wtq_t, COL_SCALE_Q, COL_SHIFT_Q,
                         COL_GATE_Q[b], q_loc, eq_loc),
                    ))
                ]
                r_tiles = {}

                def gate_mlp(pooled, col):
                    ph = ps_small.tile([P, P], F32, tag="pss")
                    for o, (c0, pc) in enumerate(CCH):
                        nc.tensor.matmul(ph[:G, 0:1], Wg1_t[:pc, o, :],
                                         pooled[:pc, o:o + 1],
                                         start=(o == 0), stop=(o == 2))
                    h = smalls.tile([G, 1], F32, tag="h")
                    nc.scalar.activation(h[:], ph[:G, 0:1], AF.Relu,
                                         bias=bg1_t[:], scale=1.0)
                    for o, (c0, pc) in enumerate(CCH):
                        pg = ps_small.tile([P, P], F32, tag="pss")
                        nc.tensor.matmul(pg[:pc, 0:1], Wg2_t[:, c0:c0 + pc], h[:])
                        nc.scalar.activation(P24[:pc, col + o:col + o + 1],
                                             pg[:pc, 0:1], AF.Sigmoid,
                                             bias=bg2_t[:pc, o:o + 1], scale=1.0)

                def load_group(g):
                    """Load the 3 residual panels of group g, reduce to pooled
                    means, and run the gate MLP."""
                    b, path, _src, _w, _sc, _sh, gcol, res_ap, _out = pans[g]
                    pooled = smalls.tile([P, 3], F32, tag="pool3")
                    nc.vector.memset(pooled[:], 0.0)
                    for o, (c0, pc) in enumerate(CCH):
                        if o < 2:
                            rt = work.tile([P, NT, 512], F32,
                                           tag=["wv_s", "wv_q"][o], name=f"rt{o}")
                        else:
                            rt = rext.tile([P, NT, 512], F32, tag="wk_r",
                                           name="rt2")
                        nc.sync.dma_start(
                            rt[:pc, :, :],
                            res_ap[b, c0:c0 + pc, :].rearrange(
                                "p (t n) -> p t n", n=512))
                        r_tiles[(g, o)] = rt
                        nc.vector.reduce_sum(pooled[:pc, o:o + 1], rt[:pc, :, :],
                                             axis=mybir.AxisListType.XY)
                    nc.vector.tensor_scalar_mul(pooled[:], pooled[:],
                                                1.0 / float(N))
                    gate_mlp(pooled, gcol)

                load_group(0)

                cc_in = dram.tile([P, 12], F32)
                cc_out = dram.tile([P, 12], F32)
                nc.gpsimd.dma_start(cc_in[:], acc[:])
                nc.gpsimd.collective_compute(
                    "AllReduce", mybir.AluOpType.add,
                    replica_groups=[list(range(NCORES))],
                    ins=[cc_in.opt()], outs=[cc_out.opt()],
                )
                cc_res = smalls.tile([P, 12], F32, tag="cc_res")
                nc.gpsimd.dma_start(cc_res[:], cc_out[:])

                def bn_coeffs(col, g_t, be_t, out_scale_col, out_shift_col, tag):
                    mean_g = smalls.tile([P, 3], F32, tag=f"mean_{tag}")
                    nc.vector.tensor_scalar_mul(mean_g[:], cc_res[:, col:col + 3],
                                                1.0 / ROWS_TOTAL)
                    var_g = smalls.tile([P, 3], F32, tag=f"var_{tag}")
                    nc.vector.tensor_scalar_mul(var_g[:],
                                                cc_res[:, col + 3:col + 6],
                                                1.0 / ROWS_TOTAL)
                    msq = smalls.tile([P, 3], F32, tag=f"msq_{tag}")
                    nc.vector.tensor_mul(msq[:], mean_g[:], mean_g[:])
                    nc.vector.tensor_sub(var_g[:], var_g[:], msq[:])
                    sd = smalls.tile([P, 3], F32, tag=f"sd_{tag}")
                    nc.scalar.activation(sd[:], var_g[:], AF.Sqrt,
                                         bias=eps_t[:], scale=1.0)
                    rstd = smalls.tile([P, 3], F32, tag=f"rstd_{tag}")
                    nc.vector.reciprocal(rstd[:], sd[:])
                    sc = P24[:, out_scale_col:out_scale_col + 3]
                    nc.vector.tensor_mul(sc, g_t[:], rstd[:])
                    tmp = smalls.tile([P, 3], F32, tag=f"shf_{tag}")
                    nc.vector.tensor_mul(tmp[:], sc, mean_g[:])
                    nc.vector.tensor_sub(P24[:, out_shift_col:out_shift_col + 3],
                                         be_t[:], tmp[:])

                bn_coeffs(0, gts_t, bets_t, COL_SCALE_P, COL_SHIFT_P, "P")
                bn_coeffs(6, gtq_t, betq_t, COL_SCALE_Q, COL_SHIFT_Q, "Q")

                for g, (b, path, src, w_base, scol, shcol, gcol, res_ap,
                        out_ap) in enumerate(pans):
                    if g + 1 < len(pans):
                        load_group(g + 1)
                    sc2 = mid.tile([P, 3], F32, tag=f"sc2_{path}_{b}")
                    sh2 = mid.tile([P, 3], F32, tag=f"sh2_{path}_{b}")
                    nc.vector.tensor_mul(sc2[:], P24[:, scol:scol + 3],
                                         P24[:, gcol:gcol + 3])
                    nc.vector.tensor_mul(sh2[:], P24[:, shcol:shcol + 3],
                                         P24[:, gcol:gcol + 3])
                    for o, (c0, pc) in enumerate(CCH):
                        r_pan = r_tiles.pop((g, o))
                        for nt in range(NT):
                            pt = ps_big.tile([P, 512], F32, tag="ps")
                            nc.tensor.matmul(pt[:pc, :], w_base[:, c0:c0 + pc],
                                             src[:, nt, :])
                            nc.scalar.activation(pt[:pc, :], pt[:pc, :],
                                                 AF.Identity,
                                                 bias=sh2[:pc, o:o + 1],
                                                 scale=sc2[:pc, o:o + 1])
                            nc.vector.tensor_add(r_pan[:pc, nt, :],
                                                 pt[:pc, :],
                                                 r_pan[:pc, nt, :])
                        nc.sync.dma_start(
                            out_ap[b, c0:c0 + pc, :].rearrange(
                                "p (t n) -> p t n", n=512),
                            r_pan[:pc, :, :])

            for _ in range(reps):
                emit_body()

    nc.compile()
    return nc


def _get_nc():
    if "nc" not in _CACHE:
        _CACHE["nc"] = build_program()
    return _CACHE["nc"]


def kernel(**inputs):
    nc = _get_nc()
    q = np.ascontiguousarray(inputs["q"], dtype=np.float32)
    s = np.ascontiguousarray(inputs["s"], dtype=np.float32)
    wnames = ["Wv", "bv", "Wk", "bk", "Wqp", "bqp", "Wts", "Wtq",
              "gts", "bets", "gtq", "betq", "Wg1", "bg1", "Wg2", "bg2"]
    weights = {k: np.ascontiguousarray(inputs[k], dtype=np.float32)
               for k in wnames}
    in_maps = []
    for c in range(NCORES):
        sl = slice(c * BPC, (c + 1) * BPC)
        in_maps.append({"q_loc": q[sl], "s_loc": s[sl], **weights})
    res = run_bass_kernel_spmd(nc, in_maps, core_ids=list(range(NCORES)))
    E_q = np.concatenate([res.results[c]["eq_loc"] for c in range(NCORES)], axis=0)
    E_s = np.concatenate([res.results[c]["es_loc"] for c in range(NCORES)], axis=0)
    return E_q, E_s
